# revision 1
# baseline (speedup 1.0000x reference)
"""Causal linear multi-head attention (decoupled phi) on 8 trn2 NeuronCores.

Sharding: core c handles batch b = c//4 and head group hg = c%4 (4 of 16 heads).
Each core computes qkv projections for its heads, chunked causal linear
attention, and a partial output projection over its 256 feature columns.
Host sums the 4 partials per batch and adds out_b.

Device layout notes:
  - activations kept feature-on-partition ("T" layouts) where matmuls need it
  - q^T,k^T: (c=512, l=2048) from W @ x^T;  v: (l, 256) from x @ Wv^T
  - k_l (l, 256) obtained from k^T by SBUF->SBUF DMA transpose (bf16)
  - per 128-chunk: A^T = K Q^T (mask upper-tri), num/den via [V|1] and [S|ksum]
  - attention output transposed back to feature-major via PE transpose for the
    output projection; partial out^T (1024, 2048) f32 DMA'd to DRAM
"""

import numpy as np
import ml_dtypes

BF = ml_dtypes.bfloat16

B, L, E, H, D = 2, 2048, 1024, 16, 64
HC = 4            # heads per core
NCORES = 8
CH = 128          # chunk length
NCH = L // CH     # 16 chunks
LTS = L // 128    # 16 l-tiles
LCH = 512         # l stripe for projections
NLC = L // LCH    # 4
ET = E // 128     # 8 e-tiles
FEPS = 1e-6
DEPS = 1e-6

PROFILE = False
_STATE = {}
import os
STAGE = int(os.environ.get("KSTAGE", "5"))
KATT = int(os.environ.get("KATT", "4"))


def _build():
    from contextlib import ExitStack
    from concourse import bacc, tile, mybir

    f32 = mybir.dt.float32
    bf16 = mybir.dt.bfloat16

    nc = bacc.Bacc("TRN2", target_bir_lowering=False, debug=False,
                   num_devices=NCORES)

    xT_d = nc.dram_tensor("xT", [E, L], bf16, kind="ExternalInput").ap()
    wqk_d = nc.dram_tensor("wqk", [E, 512], bf16, kind="ExternalInput").ap()
    bqk_d = nc.dram_tensor("bqk", [512, 1], f32, kind="ExternalInput").ap()
    wv_d = nc.dram_tensor("wv", [E, 256], bf16, kind="ExternalInput").ap()
    bv_d = nc.dram_tensor("bv", [128, 256], bf16, kind="ExternalInput").ap()
    wo_d = nc.dram_tensor("wo", [256, E], bf16, kind="ExternalInput").ap()
    mask_d = nc.dram_tensor("mask", [128, 512], f32, kind="ExternalInput").ap()
    ident_d = nc.dram_tensor("ident", [128, 128], bf16, kind="ExternalInput").ap()
    outT_d = nc.dram_tensor("outT", [E, L], bf16, kind="ExternalOutput").ap()

    with tile.TileContext(nc) as tc, ExitStack() as ctx:
        persist = ctx.enter_context(tc.tile_pool(name="persist", bufs=1))
        ps_big = ctx.enter_context(tc.tile_pool(name="psbig", bufs=3, space="PSUM"))
        ps_med = ctx.enter_context(tc.tile_pool(name="psmed", bufs=3, space="PSUM"))
        ps_st = ctx.enter_context(tc.tile_pool(name="psst", bufs=2, space="PSUM"))
        work = ctx.enter_context(tc.tile_pool(name="work", bufs=3))
        work3 = ctx.enter_context(tc.tile_pool(name="work3", bufs=3))

        def pt(shape, dt, tag):
            return persist.tile(shape, dt, tag=tag, name=tag)

        # ---- load everything ----
        xT = [pt([128, L], bf16, f"xT{i}") for i in range(ET)]
        wqk = [pt([128, 512], bf16, f"wqk{i}") for i in range(ET)]
        wv = [pt([128, 256], bf16, f"wv{i}") for i in range(ET)]
        wo = [pt([128, E], bf16, f"wo{i}") for i in range(2)]
        bqk = [pt([128, 1], f32, f"bqk{i}") for i in range(4)]
        bv = pt([128, 256], bf16, "bv")
        mask = pt([128, 512], f32, "mask")
        ident = pt([128, 128], bf16, "ident")
        # weights/x interleaved across both DMA queues so stripe-0 matmuls
        # can start as soon as the first (wqk, xT) pairs land.
        for i in range(4):
            nc.sync.dma_start(bqk[i][:], bqk_d[128 * i:128 * (i + 1), :])
        for i in range(ET):
            nc.sync.dma_start(wqk[i][:], wqk_d[128 * i:128 * (i + 1), :])
            nc.scalar.dma_start(xT[i][:], xT_d[128 * i:128 * (i + 1), :])
        for i in range(ET):
            (nc.sync if i % 2 else nc.scalar).dma_start(
                wv[i][:], wv_d[128 * i:128 * (i + 1), :])
        for i in range(2):
            nc.scalar.dma_start(wo[i][:], wo_d[128 * i:128 * (i + 1), :])
        nc.scalar.dma_start(bv[:], bv_d[:])
        nc.scalar.dma_start(mask[:], mask_d[:])
        nc.scalar.dma_start(ident[:], ident_d[:])

        # ---- persistent activations ----
        # q_blk[t]: (128, 2L) block-diagonal per chunk: cols [256c:256c+128]
        # hold the even head's q chunk on partitions 0:64 (rest zero), cols
        # [256c+128:256c+256] the odd head's on partitions 64:128. One A
        # matmul per head PAIR: lhsT = kT[t] (both heads), rhs = q_blk slab.
        q_blk = [pt([128, 2 * L], bf16, f"qblk{i}") for i in range(2)]
        kT = [pt([128, L], bf16, f"kT{i}") for i in range(2)]
        kl = [pt([128, 256], bf16, f"kl{i}") for i in range(LTS)]
        vs = [pt([128, 260], bf16, f"vs{i}") for i in range(LTS)]
        # attnT[eb][lc] : (128 e', 512 l)
        attnT = [[pt([128, LCH], bf16, f"attnT{eb}_{lc}") for lc in range(NLC)]
                 for eb in range(2)]

        # ---- phase 1: q^T / k^T projection (c-layout) ----
        # psum (128 c, 512 l) += wqk_tile[e, c].T @ xT[e, l]
        # lc outer so each stripe's dependent DMAs (odd-head shifts, k_l
        # transposes) start immediately, overlapped with later matmuls.
        qTo = [pt([64, L], bf16, f"qTo{i}") for i in range(2)]
        dmaq = [nc.sync, nc.scalar]

        # zero the blkdiag q tiles once; activations only fill diag blocks
        for t in range(2):
            nc.vector.memset(q_blk[t][:], 0.0)

        def qb3(t, part):
            # (64, NCH, 128) chunk-strided view of q_blk half `part` (0/1)
            v = q_blk[t][64 * part:64 * (part + 1), :]
            return v.rearrange("p (c w) -> p c w", w=256)[
                :, :, 128 * part:128 * (part + 1)]

        for lc in range(NLC):
            lsl = slice(LCH * lc, LCH * (lc + 1))
            cch = slice(4 * lc, 4 * (lc + 1))
            for ct in range(4):
                ps = ps_big.tile([128, LCH], f32, tag="big")
                for et in range(ET):
                    nc.tensor.matmul(
                        ps[:], wqk[et][:, 128 * ct:128 * (ct + 1)],
                        xT[et][:, LCH * lc:LCH * (lc + 1)],
                        start=(et == 0), stop=(et == ET - 1))
                if ct < 2:
                    # q: two half-activations into the blkdiag layout
                    ps3 = ps.rearrange("p (c w) -> p c w", w=128)
                    for part in range(2):
                        nc.scalar.activation(
                            qb3(ct, part)[:, cch, :],
                            ps3[64 * part:64 * (part + 1), :, :],
                            mybir.ActivationFunctionType.Relu,
                            bias=bqk[ct][64 * part:64 * (part + 1)])
                    # odd-head shift for inter matmuls (base partition 0)
                    nc.sync.dma_start(
                        qTo[ct][:, lsl].rearrange("p (c w) -> p c w", w=128),
                        qb3(ct, 1)[:, cch, :])
                else:
                    dest = kT[ct - 2]
                    nc.scalar.activation(dest[:, lsl], ps[:],
                                         mybir.ActivationFunctionType.Relu,
                                         bias=bqk[ct][:])
                if ct == 3:
                    # k_l transposes for this stripe (4 l-tiles x 2 kt)
                    for lt in range(4 * lc, 4 * (lc + 1)):
                        for kt in range(2):
                            nc.sync.dma_start_transpose(
                                kl[lt][:, 128 * kt:128 * (kt + 1)],
                                kT[kt][:, 128 * lt:128 * (lt + 1)])

        def qslab(h):
            # per-head q chunk slabs (base partition 0) for inter matmuls
            if h % 2 == 0:
                return q_blk[h // 2][0:64, :].rearrange(
                    "p (c w) -> p c w", w=256)[:, :, 0:128]
            return qTo[h // 2][:, :].rearrange("p (c w) -> p c w", w=128)

        # ---- phase 2: v projection (l-layout) + ones cols ----
        # bias comes partition-replicated from the host and is folded into
        # the psum->sbuf copy as a tensor_add (saves a matmul per l-tile)
        for lt in range(STAGE >= 2 and LTS or 0):
            ps = ps_med.tile([128, 260], f32, tag="med")
            for et in range(ET):
                nc.tensor.matmul(ps[:, 0:256],
                                 xT[et][:, 128 * lt:128 * (lt + 1)],
                                 wv[et][:], start=(et == 0), stop=(et == ET - 1))
            v3 = vs[lt].rearrange("p (h w) -> p h w", w=65)
            nc.gpsimd.memset(v3[:, :, 64:65], 1.0)
            nc.vector.tensor_add(v3[:, :, 0:64],
                                 ps[:, 0:256].rearrange("p (h w) -> p h w", w=64),
                                 bv.rearrange("p (h w) -> p h w", w=64))

        # ---- phase 4: chunked attention ----
        S = work.tile([64, 260], f32, tag="S")
        nc.vector.memset(S[:], 0.0)
        Sbf = None
        for c in range(STAGE >= 4 and NCH or 0):
            csl = slice(128 * c, 128 * (c + 1))
            # A^T per head PAIR: kT[t] holds both heads' k on its partitions,
            # q_blk's zero-padding keeps the products per-head
            psA = ps_big.tile([128, 512], f32, tag="big")
            for t in range(2):
                nc.tensor.matmul(psA[:, 256 * t:256 * (t + 1)],
                                 kT[t][:, csl],
                                 q_blk[t][:, 256 * c:256 * (c + 1)],
                                 start=True, stop=True)
            AmT = work.tile([128, 512], bf16, tag="AmT")
            nc.vector.tensor_mul(AmT[:], psA[:], mask[:])
            # state update first: PE overlaps the DVE mask mult, and Sbf for
            # chunk c+1's inter matmul is ready long before it's needed.
            Sprev = Sbf
            if KATT >= 2 and c < NCH - 1:
                psS = ps_st.tile([64, 260], f32, tag="S")
                for h in range(HC):
                    nc.tensor.matmul(psS[:, 65 * h:65 * (h + 1)],
                                     kl[c][:, 64 * h:64 * (h + 1)],
                                     vs[c][:, 65 * h:65 * (h + 1)],
                                     start=True, stop=True)
                Snew = work.tile([64, 260], f32, tag="S")
                nc.vector.tensor_add(Snew[:], S[:], psS[:])
                S = Snew
                Sbf = work.tile([64, 260], bf16, tag="Sbf")
                nc.vector.tensor_copy(Sbf[:], S[:])
            # num/den
            if KATT >= 3:
                psn = ps_med.tile([128, 260], f32, tag="med")
                for h in range(HC):
                    hsl = slice(65 * h, 65 * (h + 1))
                    nc.tensor.matmul(psn[:, hsl], AmT[:, 128 * h:128 * (h + 1)],
                                     vs[c][:, hsl], start=True, stop=(c == 0))
                    if c > 0:
                        nc.tensor.matmul(psn[:, hsl], qslab(h)[:, c, :],
                                         Sprev[:, hsl], start=False, stop=True)
                # dens -> reciprocal
                d4 = work.tile([128, 4], f32, tag="d4")
                nc.vector.tensor_scalar_max(
                    d4[:],
                    psn.rearrange("p (h w) -> p h w", w=65)[:, :, 64:65].opt(),
                    DEPS)
                r4 = work.tile([128, 4], f32, tag="r4")
                nc.vector.reciprocal(r4[:], d4[:])
                att = work.tile([128, 256], bf16, tag="att")
                for h in range(HC):
                    nc.vector.tensor_scalar_mul(
                        att[:, 64 * h:64 * (h + 1)],
                        psn[:, 65 * h:65 * h + 64], r4[:, h:h + 1])
            # transpose attention chunk to feature-major
            if KATT >= 4:
                psT = ps_med.tile([128, 256], bf16, tag="med")
                for eb in range(2):
                    nc.tensor.transpose(psT[:, 128 * eb:128 * (eb + 1)],
                                        att[:, 128 * eb:128 * (eb + 1)], ident[:])
                    nc.scalar.copy(
                        attnT[eb][c // 4][:, 128 * (c % 4):128 * (c % 4 + 1)],
                        psT[:, 128 * eb:128 * (eb + 1)])

        # ---- phase 5: output projection ----
        for lc in range(STAGE >= 5 and NLC or 0):
            for ot in range(ET):
                ps = ps_big.tile([128, LCH], f32, tag="big")
                for eb in range(2):
                    nc.tensor.matmul(ps[:], wo[eb][:, 128 * ot:128 * (ot + 1)],
                                     attnT[eb][lc][:],
                                     start=(eb == 0), stop=(eb == 1))
                ob = work3.tile([128, LCH], bf16, tag="ob")
                nc.vector.tensor_copy(ob[:], ps[:])
                dmaq[(lc * ET + ot) % 2].dma_start(
                    outT_d[128 * ot:128 * (ot + 1), LCH * lc:LCH * (lc + 1)],
                    ob[:])

    nc.compile()
    return nc


def _prep_inputs(x, qkv_w, qkv_b, out_w):
    mask = np.tile(np.triu(np.ones((128, 128), np.float32)), (1, 4))
    ident = np.eye(128, dtype=np.float32).astype(BF)
    in_maps = []
    for c in range(NCORES):
        b, hg = c // 4, c % 4
        rows = np.arange(256 * hg, 256 * (hg + 1))
        wqk = np.concatenate([qkv_w[rows], qkv_w[rows + E]], 0).T
        bqk = np.concatenate([qkv_b[rows], qkv_b[rows + E]])[:, None]
        wv = qkv_w[rows + 2 * E].T
        bv = np.tile(qkv_b[rows + 2 * E][None, :], (128, 1))
        wo = out_w[:, rows].T
        in_maps.append({
            "xT": np.ascontiguousarray(x[b].T).astype(BF),
            "wqk": np.ascontiguousarray(wqk).astype(BF),
            "bqk": np.ascontiguousarray(bqk).astype(np.float32),
            "wv": np.ascontiguousarray(wv).astype(BF),
            "bv": np.ascontiguousarray(bv).astype(BF),
            "wo": np.ascontiguousarray(wo).astype(BF),
            "mask": mask, "ident": ident,
        })
    return in_maps


def kernel(x, qkv_w, qkv_b, out_w, out_b):
    from concourse.bass_utils import run_bass_kernel_spmd

    x = np.asarray(x, np.float32)
    qkv_w = np.asarray(qkv_w, np.float32)
    qkv_b = np.asarray(qkv_b, np.float32)
    out_w = np.asarray(out_w, np.float32)
    out_b = np.asarray(out_b, np.float32)

    if "nc" not in _STATE:
        _STATE["nc"] = _build()
    nc = _STATE["nc"]
    in_maps = _prep_inputs(x, qkv_w, qkv_b, out_w)
    res = run_bass_kernel_spmd(nc, in_maps, list(range(NCORES)),
                               trace=PROFILE)
    _STATE["last"] = res
    out = np.zeros((B, L, E), np.float32)
    for c in range(NCORES):
        out[c // 4] += res.results[c]["outT"].T
    out += out_b
    return out

